# revision 8
# baseline (speedup 1.0000x reference)
"""Trainium2 Bass kernel for nn_EdgeModel (GNN edge-MLP message passing).

Reference computation (per edge e):
    h = concat([x_s[src[e]], x_t[tgt[e]], edge_attr[e], u[batch_e[e]]])  # [512]
    h = leaky_relu(h @ W1 + b1, 0.1)                                     # [128]
    out[e] = h @ W2 + b2                                                 # [128]

Sharding: data-parallel over edges across 8 cores; x_s/x_t and weights
replicated, edge arrays split into per-core chunks.

Gather strategy: the node tables are too large for int16 indexing, so each
core's edges are sorted by (src_slab, tgt_slab) with slabs of 32768 rows.
That yields <=16 contiguous segments per core within which both gathers read
from a fixed table slab using slab-relative int16 indices, served by the
high-throughput InstDMAGatherAnt (one instruction per segment x super-tile,
~0.34ns/row of GpSimd descriptor time vs ~1us/128 rows for generic indirect
DMA). Segment sizes are padded to multiples of 128 slots and made uniform
across cores so all 8 cores share one SPMD program.

Edge slot layout: position i -> (partition p=i%128, column g=(i%2048)//128)
within super-tile st=i//2048 (dma_gather's native placement). edge_attr and
out rows are host-permuted so the device DMAs stay 8KB-contiguous per
partition.

Device dataflow per 512-edge block: PE-transpose the three gathered chunks
to [feat, edge] layout, accumulate 4 matmuls into PSUM h1T [128, 512] (the
u@W1u+b1 term uses a one-hot selection matrix built on-chip from batch ids
against a host-precomputed U1 = u@W1u + b1 table), LeakyReLU via
max(x, 0.1x), second matmul, add b2, PE-transpose back, store.
"""
import numpy as np

import concourse.bass as bass
import concourse.mybir as mybir
import concourse.tile as tile
from concourse import bacc
from concourse.bass_utils import run_bass_kernel_spmd
from concourse.masks import make_identity

fp = mybir.dt.float32
bf = mybir.dt.bfloat16
i16 = mybir.dt.int16

P = 128            # partitions
D = 128            # feature dim per chunk
B = 64             # global batches
N_CORES = 8

G = 16             # columns (128-slot groups) per super-tile
SUPER = P * G      # 2048 edge slots per super-tile
JBLK = 4           # columns per compute block
BLK = P * JBLK     # 512 edges per compute block
NBLK = G // JBLK   # blocks per super-tile

N_NODES = 100000
E_TOTAL = 500000
SLAB = 32768       # int16-addressable node-table slab


def build_kernel(src_calls, tgt_calls, n_super, n_nodes=N_NODES):
    """src_calls/tgt_calls: list of (lo, hi, base) slot ranges (lo/hi multiples
    of 128, within one super-tile each) gathering table[base:...] rows."""
    e_pad = n_super * SUPER
    ncols = n_super * G

    calls_by_super = {"s": {}, "t": {}}
    for key, calls in (("s", src_calls), ("t", tgt_calls)):
        for lo, hi, base in calls:
            assert lo % P == 0 and hi % P == 0 and lo // SUPER == (hi - 1) // SUPER
            calls_by_super[key].setdefault(lo // SUPER, []).append((lo, hi, base))

    nc = bacc.Bacc("TRN2", target_bir_lowering=False, debug=False)
    x_s = nc.dram_tensor("x_s", [n_nodes, D], bf, kind="ExternalInput")
    x_t = nc.dram_tensor("x_t", [n_nodes, D], bf, kind="ExternalInput")
    ea = nc.dram_tensor("ea", [e_pad, D], bf, kind="ExternalInput")
    src_t = nc.dram_tensor("src_t", [P, e_pad // 16], i16, kind="ExternalInput")
    tgt_t = nc.dram_tensor("tgt_t", [P, e_pad // 16], i16, kind="ExternalInput")
    bat_t = nc.dram_tensor("bat_t", [P, ncols], fp, kind="ExternalInput")
    W1s = nc.dram_tensor("W1s", [D, D], bf, kind="ExternalInput")
    W1t = nc.dram_tensor("W1t", [D, D], bf, kind="ExternalInput")
    W1e = nc.dram_tensor("W1e", [D, D], bf, kind="ExternalInput")
    U1 = nc.dram_tensor("U1", [B, D], bf, kind="ExternalInput")
    W2 = nc.dram_tensor("W2", [D, D], bf, kind="ExternalInput")
    b2 = nc.dram_tensor("b2", [D, 1], fp, kind="ExternalInput")
    iota = nc.dram_tensor("iota", [B, 1], fp, kind="ExternalInput")
    out = nc.dram_tensor("out", [e_pad, D], fp, kind="ExternalOutput")

    ea_r = ea[:].rearrange("(s p g) f -> s p g f", p=P, g=G)
    out_r = out[:].rearrange("(s p g) f -> s p g f", p=P, g=G)

    with tile.TileContext(nc) as tc:
        with (
            tc.tile_pool(name="const", bufs=1) as cpool,
            tc.tile_pool(name="gath", bufs=2) as gpool,
            tc.tile_pool(name="osb", bufs=2) as opool,
            tc.tile_pool(name="blk", bufs=3) as bpool,
            tc.tile_pool(name="ps_acc", bufs=1, space="PSUM") as ps_acc,
            tc.tile_pool(name="ps_tr", bufs=2, space="PSUM") as ps_tr,
            tc.tile_pool(name="ps_out", bufs=2, space="PSUM") as ps_out,
        ):
            ident = cpool.tile([P, P], fp)
            make_identity(nc, ident[:])
            ident_bf = cpool.tile([P, P], bf)
            nc.vector.tensor_copy(out=ident_bf[:], in_=ident[:])
            w1s_t = cpool.tile([D, D], bf)
            nc.sync.dma_start(out=w1s_t[:], in_=W1s[:])
            w1t_t = cpool.tile([D, D], bf)
            nc.sync.dma_start(out=w1t_t[:], in_=W1t[:])
            w1e_t = cpool.tile([D, D], bf)
            nc.sync.dma_start(out=w1e_t[:], in_=W1e[:])
            u1_t = cpool.tile([B, D], bf)
            nc.sync.dma_start(out=u1_t[:], in_=U1[:])
            w2_t = cpool.tile([D, D], bf)
            nc.sync.dma_start(out=w2_t[:], in_=W2[:])
            b2_t = cpool.tile([D, 1], fp)
            nc.sync.dma_start(out=b2_t[:], in_=b2[:])
            iota_t = cpool.tile([B, 1], fp)
            nc.sync.dma_start(out=iota_t[:], in_=iota[:])
            sidx = cpool.tile([P, e_pad // 16], i16)
            nc.sync.dma_start(out=sidx[:], in_=src_t[:])
            tidx = cpool.tile([P, e_pad // 16], i16)
            nc.sync.dma_start(out=tidx[:], in_=tgt_t[:])
            bval = cpool.tile([P, ncols], fp)
            nc.sync.dma_start(out=bval[:], in_=bat_t[:])

            MAX_IDX_PER_CALL = 1024  # keep per-engine descriptor ring <= 64

            def gathers(st, key, table_ap, idx_tile, out_tile):
                for lo0, hi0, base in calls_by_super[key].get(st, []):
                    nrows = min(SLAB, n_nodes - base)
                    for lo in range(lo0, hi0, MAX_IDX_PER_CALL):
                        hi = min(hi0, lo + MAX_IDX_PER_CALL)
                        n = hi - lo
                        g0 = (lo % SUPER) // P
                        nc.gpsimd.dma_gather(
                            out_ap=out_tile[:, g0:g0 + n // P, :],
                            in_ap=table_ap[base:base + nrows, :],
                            idxs_ap=idx_tile[:, lo // 16:hi // 16],
                            num_idxs=n, num_idxs_reg=n, elem_size=D,
                            single_packet=False)

            for st in range(n_super):
                hs = gpool.tile([P, G, D], bf, tag="hs")
                gathers(st, "s", x_s, sidx, hs)
                ht = gpool.tile([P, G, D], bf, tag="ht")
                gathers(st, "t", x_t, tidx, ht)
                ea_tile = gpool.tile([P, G, D], bf, tag="ea")
                nc.sync.dma_start(out=ea_tile[:], in_=ea_r[st])

                out_sb = opool.tile([P, G, D], fp, tag="out")

                for b in range(NBLK):
                    # transpose chunks into [feat, c] layout, c = j*128 + p
                    hsT = bpool.tile([D, BLK], bf, tag="hsT")
                    htT = bpool.tile([D, BLK], bf, tag="htT")
                    eaT = bpool.tile([D, BLK], bf, tag="eaT")
                    for ci, (tin, tout) in enumerate(
                            ((hs, hsT), (ht, htT), (ea_tile, eaT))):
                        pt = ps_tr.tile([P, BLK], bf, tag="tr")
                        for j in range(JBLK):
                            nc.tensor.transpose(
                                out=pt[:, j * P:(j + 1) * P],
                                in_=tin[:, JBLK * b + j, :],
                                identity=ident_bf[:])
                        if ci == 2:  # balance: eaT copy on ScalarE
                            nc.scalar.copy(out=tout[:], in_=pt[:])
                        else:
                            nc.vector.tensor_copy(out=tout[:], in_=pt[:])

                    # one-hot selection matrix from batch ids
                    ptb = ps_tr.tile([B, BLK], fp, tag="trb")
                    for j in range(JBLK):
                        col = st * G + JBLK * b + j
                        nc.tensor.transpose(
                            out=ptb[:, j * P:(j + 1) * P],
                            in_=bval[:, col:col + 1].to_broadcast([P, B]),
                            identity=ident[:])
                    selT = bpool.tile([B, BLK], bf, tag="selT")
                    nc.vector.tensor_tensor(
                        out=selT[:], in0=iota_t[:].to_broadcast([B, BLK]),
                        in1=ptb[:], op=mybir.AluOpType.is_equal)

                    # layer 1: h1T[f1, c]
                    h1T = ps_acc.tile([D, BLK], fp, tag="h1T")
                    nc.tensor.matmul(out=h1T[:], lhsT=w1s_t[:], rhs=hsT[:],
                                     start=True, stop=False)
                    nc.tensor.matmul(out=h1T[:], lhsT=w1t_t[:], rhs=htT[:],
                                     start=False, stop=False)
                    nc.tensor.matmul(out=h1T[:], lhsT=w1e_t[:], rhs=eaT[:],
                                     start=False, stop=False)
                    nc.tensor.matmul(out=h1T[:], lhsT=u1_t[:], rhs=selT[:],
                                     start=False, stop=True)

                    # LeakyReLU(0.1) = max(x, 0.1x)
                    t_sb = bpool.tile([D, BLK], fp, tag="t")
                    nc.scalar.activation(
                        out=t_sb[:], in_=h1T[:],
                        func=mybir.ActivationFunctionType.Copy, scale=0.1)
                    aT = bpool.tile([D, BLK], bf, tag="aT")
                    nc.vector.tensor_tensor(out=aT[:], in0=t_sb[:], in1=h1T[:],
                                            op=mybir.AluOpType.max)

                    # layer 2 + b2
                    o2T = ps_acc.tile([D, BLK], fp, tag="o2T")
                    nc.tensor.matmul(out=o2T[:], lhsT=w2_t[:], rhs=aT[:],
                                     start=True, stop=True)
                    o2s = bpool.tile([D, BLK], fp, tag="o2s")
                    nc.scalar.activation(
                        out=o2s[:], in_=o2T[:],
                        func=mybir.ActivationFunctionType.Identity,
                        bias=b2_t[:, :1])

                    # transpose back to [edge, feat] layout
                    pto = ps_out.tile([P, BLK], fp, tag="tro")
                    for j in range(JBLK):
                        nc.tensor.transpose(
                            out=pto[:, j * P:(j + 1) * P],
                            in_=o2s[:, j * P:(j + 1) * P], identity=ident[:])
                    nc.vector.tensor_copy(
                        out=out_sb[:, JBLK * b:JBLK * (b + 1), :], in_=pto[:])

                nc.sync.dma_start(out=out_r[st], in_=out_sb[:])

    nc.compile()
    return nc


def _plan_segments(edge_index, batch_e, edge_attr, n_nodes=N_NODES):
    """Sort each core's edges by (src_slab, tgt_slab); uniform segment sizes
    across cores (padded to 128 slots). Returns per-core position-ordered
    arrays, slot->original-edge maps, call lists, and n_super."""
    e_core = E_TOTAL // N_CORES
    src = np.asarray(edge_index[0])
    tgt = np.asarray(edge_index[1])
    n_slab_s = -(-n_nodes // SLAB)
    n_slab_t = n_slab_s

    per_core = []
    counts = np.zeros((N_CORES, n_slab_s, n_slab_t), np.int64)
    for c in range(N_CORES):
        sl = slice(c * e_core, (c + 1) * e_core)
        s, t = src[sl], tgt[sl]
        key = (s // SLAB) * n_slab_t + (t // SLAB)
        order = np.argsort(key, kind="stable")
        per_core.append(order)
        cnt = np.bincount(key, minlength=n_slab_s * n_slab_t)
        counts[c] = cnt.reshape(n_slab_s, n_slab_t)

    seg_sizes = (-(-counts.max(axis=0) // P)) * P      # [ns, nt] multiples of 128
    total = int(seg_sizes.sum())
    n_super = -(-total // SUPER)
    e_pad = n_super * SUPER

    # segment start offsets (position space), row-major over (s_slab, t_slab)
    starts = np.zeros_like(seg_sizes)
    acc = 0
    seg_list = []
    for i in range(n_slab_s):
        for j in range(n_slab_t):
            starts[i, j] = acc
            if seg_sizes[i, j]:
                seg_list.append((i, j, acc, acc + int(seg_sizes[i, j])))
            acc += int(seg_sizes[i, j])

    # gather calls: split by super-tile boundaries; src merges contiguous
    # same-src-slab segments
    def split_ranges(ranges):
        calls = []
        for lo, hi, base in ranges:
            while lo < hi:
                hi2 = min(hi, (lo // SUPER + 1) * SUPER)
                calls.append((lo, hi2, base))
                lo = hi2
        return calls

    src_ranges = []
    for i in range(n_slab_s):
        lo = int(starts[i, 0])
        hi = int(starts[i, n_slab_t - 1] + seg_sizes[i, n_slab_t - 1])
        if hi > lo:
            src_ranges.append((lo, hi, i * SLAB))
    # tail beyond last segment: pad slots gather from slab 0
    if acc < e_pad:
        src_ranges.append((acc, e_pad, 0))
    tgt_ranges = [(lo, hi, j * SLAB) for (i, j, lo, hi) in seg_list]
    if acc < e_pad:
        tgt_ranges.append((acc, e_pad, 0))
    src_calls = split_ranges(src_ranges)
    tgt_calls = split_ranges(tgt_ranges)
    return per_core, counts, seg_sizes, starts, n_super, src_calls, tgt_calls


def _host_prep(inputs):
    import ml_dtypes
    bf_np = ml_dtypes.bfloat16
    x_s = np.ascontiguousarray(np.asarray(inputs["x_s"]).astype(bf_np))
    x_t = np.ascontiguousarray(np.asarray(inputs["x_t"]).astype(bf_np))
    edge_index = np.asarray(inputs["edge_index"])
    edge_attr = np.asarray(inputs["edge_attr"], dtype=np.float32)
    u = np.asarray(inputs["u"], dtype=np.float32)
    batch_e = np.asarray(inputs["batch_e"])
    W1 = np.asarray(inputs["W1"], dtype=np.float32)
    b1 = np.asarray(inputs["b1"], dtype=np.float32)
    W2 = np.asarray(inputs["W2"], dtype=np.float32)
    b2 = np.asarray(inputs["b2"], dtype=np.float32)

    (per_core_order, counts, seg_sizes, starts, n_super,
     src_calls, tgt_calls) = _plan_segments(edge_index, batch_e, edge_attr)
    e_pad = n_super * SUPER
    ncols = n_super * G
    e_core = E_TOTAL // N_CORES

    U1 = np.ascontiguousarray((u @ W1[384:512] + b1).astype(bf_np))
    shared = {
        "x_s": x_s, "x_t": x_t,
        "W1s": np.ascontiguousarray(W1[0:128].astype(bf_np)),
        "W1t": np.ascontiguousarray(W1[128:256].astype(bf_np)),
        "W1e": np.ascontiguousarray(W1[256:384].astype(bf_np)),
        "U1": U1, "W2": np.ascontiguousarray(W2.astype(bf_np)),
        "b2": np.ascontiguousarray(b2.reshape(D, 1)),
        "iota": np.arange(B, dtype=np.float32).reshape(B, 1),
    }

    def wrap16(vals):
        w = vals.reshape(-1, 16).T                     # [16, e_pad/16]
        return np.ascontiguousarray(np.tile(w, (8, 1)))

    n_slab_t = seg_sizes.shape[1]
    in_maps, perms = [], []
    for c in range(N_CORES):
        sl = slice(c * e_core, (c + 1) * e_core)
        order = per_core_order[c]
        s = edge_index[0, sl][order]
        t = edge_index[1, sl][order]
        bat = batch_e[sl][order]
        eat = edge_attr[sl][order]

        # place sorted edges into the uniform segment skeleton
        pos = np.zeros(e_pad, np.int64)          # position -> sorted-edge id+1
        ofs = 0
        for i in range(seg_sizes.shape[0]):
            for j in range(n_slab_t):
                n = counts[c, i, j]
                st0 = int(starts[i, j])
                pos[st0:st0 + n] = np.arange(ofs, ofs + n) + 1
                ofs += n
        valid = pos > 0
        src_pos = np.zeros(e_pad, np.int64)
        tgt_pos = np.zeros(e_pad, np.int64)
        bat_pos = np.zeros(e_pad, np.int64)
        ea_pos = np.zeros((e_pad, D), bf_np)
        idx = pos[valid] - 1
        src_pos[valid] = s[idx]
        tgt_pos[valid] = t[idx]
        bat_pos[valid] = bat[idx]
        ea_pos[valid] = eat[idx]
        # slab-relative int16 (padding slots stay 0 within their slab)
        s16 = (src_pos % SLAB).astype(np.int16)
        t16 = (tgt_pos % SLAB).astype(np.int16)

        # permute position-ordered rows to the device p-major DRAM layout:
        # DRAM row st*2048 + p*16 + g <- position st*2048 + g*128 + p
        def pos_to_dram(a):
            return np.ascontiguousarray(
                a.reshape(n_super, G, P, -1).transpose(0, 2, 1, 3)
                .reshape(e_pad, -1).squeeze())

        bat_tab = (bat_pos.reshape(n_super, G, P).transpose(2, 0, 1)
                   .reshape(P, ncols).astype(np.float32))
        in_maps.append({
            **shared,
            "ea": pos_to_dram(ea_pos).reshape(e_pad, D),
            "src_t": wrap16(s16), "tgt_t": wrap16(t16),
            "bat_t": np.ascontiguousarray(bat_tab),
        })
        # slot position of original edge k (for output unpermute)
        inv = np.zeros(e_core, np.int64)
        inv[order] = np.arange(e_core)
        pos_of_sorted = np.zeros(e_core, np.int64)
        pos_of_sorted[pos[valid] - 1] = np.where(valid)[0]
        perms.append(pos_of_sorted[inv])
    return in_maps, perms, n_super, src_calls, tgt_calls


_NC_CACHE = {}


def kernel(**inputs) -> np.ndarray:
    in_maps, perms, n_super, src_calls, tgt_calls = _host_prep(inputs)
    key = (n_super, tuple(src_calls), tuple(tgt_calls))
    if key not in _NC_CACHE:
        _NC_CACHE.clear()
        _NC_CACHE[key] = build_kernel(src_calls, tgt_calls, n_super)
    nc = _NC_CACHE[key]
    res = run_bass_kernel_spmd(nc, in_maps, core_ids=list(range(N_CORES)))
    e_core = E_TOTAL // N_CORES
    outs = []
    for c in range(N_CORES):
        # device DRAM row st*2048+p*16+g holds position st*2048+g*128+p
        o = res.results[c]["out"].reshape(n_super, P, G, D).transpose(0, 2, 1, 3)
        outs.append(o.reshape(n_super * SUPER, D)[perms[c]])
    return np.concatenate(outs, axis=0)
